# revision 8
# baseline (speedup 1.0000x reference)
# Cost-volume concatenation kernel for Trainium2 (Bass/Tile), SPMD over 8 cores.
#
# Problem: left, right: [B=2, H=64, W=256, C=32] f32.
# out[b, d+48, h, w, :32] = left[b,h,w,:]  * valid(w,d)
# out[b, d+48, h, w, 32:] = right[b,h,w-d,:] * valid(w,d),  d in [-48, 48)
# valid(w,d) = 0 <= w-d < W.  Output [2, 96, 64, 256, 64] f32 (~805 MB).
#
# Sharding: disparity axis. Core k handles the 12 levels d in [12k-48, 12k-36).
# The kernel program is identical on every core; all per-core variation lives in
# the DATA:
#   - rpad:  right pre-shifted by the core's base disparity and zero-padded to
#            width TPAD, so the in-kernel shift is j in [0,12) for every core and
#            the zero padding implements the right-half validity masking.
#   - vrep:  a 0/1 validity mask with the same index structure, replicated
#            across the 128 SBUF partitions; out_left = left * vrep_shifted
#            implements the left-half masking.
#
# SBUF layout: partitions = (h, b) — h-major — p = 2*h + b, 128 partitions;
# free dim = (w, c). h-major matters: the output DMA's DRAM access pattern is
# then [h=64, b=2, wc] with outer dim 64, which HWDGE fans out across all 16
# SDMA engines. (A b-major [2, 64, wc] pattern splits over only 2 engines ->
# ~27 GB/s per core; SWDGE spreads by partition but its descriptor ring
# backpressure caps concurrency at ~4 engines for multi-descriptor transfers.)
#
# Per disparity j the kernel assembles interleaved [left|right] rows in SBUF
# (two tensor ops per w-chunk) and streams them out with contiguous HWDGE DMAs.
#
# Precision: the whole pipeline runs in bf16 (inputs are rounded to bf16 on
# host; the mask is exact 0/1 and the copy/mul are exact in bf16, so the only
# error is the input rounding, ~1e-3 rel — far inside the 2e-2 gate). This
# halves DMA traffic vs f32: ~4.4 MB read + ~50 MB write per core
# (memory-bound; the f32 version measured 270us, bound at ~405 GB/s/core).

import ml_dtypes
import numpy as np

B, H, W, C = 2, 64, 256, 32
MAX_DISP = 48
D2 = 2 * MAX_DISP            # 96 disparity levels
N_CORES = 8
DPC = D2 // N_CORES          # 12 disparities per core
JPAD = DPC - 1               # 11: shift offset so in-kernel shifts are >= 0
TPAD = 272                   # padded t-width (>= W + JPAD = 267)
P = B * H                    # 128 SBUF partitions = (h, b) h-major
WC = W * C                   # 8192
TC = TPAD * C                # 8704
WCHUNK = 128                 # w-columns per output tile / DMA (2 MB per DMA)
F32 = np.float32
BF16 = ml_dtypes.bfloat16

_CACHE = {}


def _build_nc():
    import concourse.bacc as bacc
    import concourse.mybir as mybir
    from concourse.tile import TileContext, add_dep_helper

    bf16 = mybir.dt.bfloat16
    nc = bacc.Bacc("TRN2", target_bir_lowering=False, debug=False)
    left_t = nc.dram_tensor("left_flat", [P, WC], bf16, kind="ExternalInput")
    rpad_t = nc.dram_tensor("rpad", [P, TC], bf16, kind="ExternalInput")
    vrep_t = nc.dram_tensor("vrep", [P, TPAD], bf16, kind="ExternalInput")
    out_t = nc.dram_tensor("out", [B, DPC, H, W * 2 * C], bf16, kind="ExternalOutput")
    # DMA-side view iterating (j, h, b, cols): outer dim 64 for 16-way fan-out.
    out_perm = out_t.ap().rearrange("b j h m -> j h b m")

    with TileContext(nc) as tc:
        with (
            tc.tile_pool(name="ins", bufs=1) as ipool,
            tc.tile_pool(name="outs", bufs=3) as opool,
        ):
            left_sb = ipool.tile([P, WC], bf16, tag="left")
            rpad_sb = ipool.tile([P, TC], bf16, tag="rpad")
            vrep_sb = ipool.tile([P, TPAD], bf16, tag="vrep")
            # Phased input loads: the head (~4.4 MB) drains alone at full read
            # bandwidth so the first output DMA can start ~20us in; the tail
            # halves are gated to drain underneath the first output DMAs
            # (without the gate, all loads round-robin on the shared SDMA
            # engines at packet granularity and the head finishes no earlier
            # than the whole input set). vrep is one mask value per t column
            # (139 KB total) — the mul broadcasts it across the 32 channels
            # with a step-0 inner AP dim.
            SPLIT_L = WCHUNK * C  # left head: w < 128 (everything wi=0 needs)
            SPLIT_R = 144 * C     # rpad head: t < 144 (wi=0 outputs read t < 140)
            head = [
                nc.sync.dma_start(out=vrep_sb[:], in_=vrep_t[:]),
                nc.sync.dma_start(out=left_sb[:, :SPLIT_L], in_=left_t[:, :SPLIT_L]),
                nc.sync.dma_start(out=rpad_sb[:, :SPLIT_R], in_=rpad_t[:, :SPLIT_R]),
            ]
            tail = [
                nc.scalar.dma_start(out=left_sb[:, SPLIT_L:], in_=left_t[:, SPLIT_L:]),
                nc.scalar.dma_start(out=rpad_sb[:, SPLIT_R:], in_=rpad_t[:, SPLIT_R:]),
            ]
            for t_ in tail:
                for h_ in head:
                    add_dep_helper(
                        t_.ins, h_.ins,
                        reason="input tail loads drain after head loads",
                    )

            lv = left_sb[:].rearrange("p (w c) -> p w c", c=C)
            rv = rpad_sb[:].rearrange("p (t c) -> p t c", c=C)
            vv = vrep_sb[:]  # [p, t]; broadcast across c inside the mul

            for wi in range(0, W, WCHUNK):
                for j in reversed(range(DPC)):
                    ot = opool.tile([P, WCHUNK * 2 * C], bf16, tag="ot")
                    ov = ot[:].rearrange("p (w c) -> p w c", c=2 * C)
                    t0 = wi + JPAD - j
                    nc.vector.tensor_mul(
                        out=ov[:, :, 0:C],
                        in0=lv[:, wi : wi + WCHUNK, :],
                        in1=vv[:, t0 : t0 + WCHUNK, None].broadcast_to(
                            [P, WCHUNK, C]
                        ),
                    )
                    nc.vector.tensor_copy(
                        out=ov[:, :, C : 2 * C],
                        in_=rv[:, t0 : t0 + WCHUNK, :],
                    )
                    nc.sync.dma_start(
                        out=out_perm[j, :, :, wi * 2 * C : (wi + WCHUNK) * 2 * C],
                        in_=ot[:],
                    )
    nc.finalize()
    return nc


def get_nc():
    if "nc" not in _CACHE:
        _CACHE["nc"] = _build_nc()
    return _CACHE["nc"]


def _hb_major(x):
    """[B, H, rest...] -> [128 = (h, b) h-major, prod(rest)] contiguous."""
    return np.ascontiguousarray(x.transpose(1, 0, 2, 3)).reshape(P, -1)


def prep_inputs(left, right):
    """Build the 8 per-core input maps from full left/right."""
    left = np.asarray(left, dtype=F32).astype(BF16)
    right = np.asarray(right, dtype=F32).astype(BF16)
    left_flat = _hb_major(left)
    in_maps = []
    for k in range(N_CORES):
        d0 = DPC * k - MAX_DISP
        shift = JPAD + d0        # rpad[..., t, :] = right[..., t - shift, :]
        rpad = np.zeros((B, H, TPAD, C), BF16)
        lo, hi = max(0, shift), min(TPAD, shift + W)
        if lo < hi:
            rpad[:, :, lo:hi, :] = right[:, :, lo - shift : hi - shift, :]
        vk = np.zeros(TPAD, BF16)
        vk[lo:hi] = 1.0
        vrep = np.ascontiguousarray(np.broadcast_to(vk, (P, TPAD)))
        in_maps.append(
            {"left_flat": left_flat, "rpad": _hb_major(rpad), "vrep": vrep}
        )
    return in_maps


def run(left, right, **kwargs):
    """Run the SPMD kernel; returns (full_output, BassKernelResults)."""
    from concourse.bass_utils import run_bass_kernel_spmd

    nc = get_nc()
    in_maps = prep_inputs(left, right)
    try:
        res = run_bass_kernel_spmd(
            nc, in_maps, core_ids=list(range(N_CORES)), **kwargs
        )
    except Exception:
        # The axon/neuron device occasionally reports a transient
        # NRT_EXEC_UNIT_UNRECOVERABLE on a cold first run; a retry succeeds.
        res = run_bass_kernel_spmd(
            nc, in_maps, core_ids=list(range(N_CORES)), **kwargs
        )
    full = np.concatenate(
        [r["out"].reshape(B, DPC, H, W, 2 * C) for r in res.results], axis=1
    ).astype(np.float32)
    return full, res


def kernel(left, right):
    full, _ = run(left, right)
    return full



# revision 14
# speedup vs baseline: 1.1573x; 1.1573x over previous
# Cost-volume concatenation kernel for Trainium2 (Bass/Tile), SPMD over 8 cores.
#
# Problem: left, right: [B=2, H=64, W=256, C=32] f32.
# out[b, d+48, h, w, :32] = left[b,h,w,:]  * valid(w,d)
# out[b, d+48, h, w, 32:] = right[b,h,w-d,:] * valid(w,d),  d in [-48, 48)
# valid(w,d) = 0 <= w-d < W.  Output [2, 96, 64, 256, 64] f32 (~805 MB).
#
# Sharding: disparity axis. Core k handles the 12 levels d in [12k-48, 12k-36).
# The kernel program is identical on every core; all per-core variation lives in
# the DATA:
#   - rpad:  right pre-shifted by the core's base disparity and zero-padded to
#            width TPAD, so the in-kernel shift is j in [0,12) for every core and
#            the zero padding implements the right-half validity masking.
#   - vrep:  a 0/1 validity mask with the same index structure, replicated
#            across the 128 SBUF partitions; out_left = left * vrep_shifted
#            implements the left-half masking.
#
# SBUF layout: partitions = (h, b) — h-major — p = 2*h + b, 128 partitions;
# free dim = (w, c). h-major matters: the output DMA's DRAM access pattern is
# then [h=64, b=2, wc] with outer dim 64, which HWDGE fans out across all 16
# SDMA engines. (A b-major [2, 64, wc] pattern splits over only 2 engines ->
# ~27 GB/s per core; SWDGE spreads by partition but its descriptor ring
# backpressure caps concurrency at ~4 engines for multi-descriptor transfers.)
#
# Per disparity j the kernel assembles interleaved [left|right] rows in SBUF
# (two tensor ops per w-chunk) and streams them out with contiguous HWDGE DMAs.
#
# Precision: the whole pipeline runs in bf16 (inputs are rounded to bf16 on
# host; the mask is exact 0/1 and the copy/mul are exact in bf16, so the only
# error is the input rounding, ~1e-3 rel — far inside the 2e-2 gate). This
# halves DMA traffic vs f32: ~4.4 MB read + ~50 MB write per core
# (memory-bound; the f32 version measured 270us, bound at ~405 GB/s/core).

import ml_dtypes
import numpy as np

B, H, W, C = 2, 64, 256, 32
MAX_DISP = 48
D2 = 2 * MAX_DISP            # 96 disparity levels
N_CORES = 8
DPC = D2 // N_CORES          # 12 disparities per core
JPAD = DPC - 1               # 11: shift offset so in-kernel shifts are >= 0
TPAD = 272                   # padded t-width (>= W + JPAD = 267)
P = B * H                    # 128 SBUF partitions = (h, b) h-major
WC = W * C                   # 8192
TC = TPAD * C                # 8704
WCHUNK = 128                 # w-columns per output tile / DMA (2 MB per DMA)
F32 = np.float32
BF16 = ml_dtypes.bfloat16

_CACHE = {}


def _build_nc():
    import concourse.bacc as bacc
    import concourse.mybir as mybir
    from concourse.tile import TileContext, add_dep_helper

    bf16 = mybir.dt.bfloat16
    nc = bacc.Bacc("TRN2", target_bir_lowering=False, debug=False)
    left_t = nc.dram_tensor("left_flat", [P, WC], bf16, kind="ExternalInput")
    rpad_t = nc.dram_tensor("rpad", [P, TC], bf16, kind="ExternalInput")
    vrep_t = nc.dram_tensor("vrep", [P, TC], bf16, kind="ExternalInput")
    out_t = nc.dram_tensor("out", [B, DPC, H, W * 2 * C], bf16, kind="ExternalOutput")
    # DMA-side view iterating (j, h, b, cols): outer dim 64 for 16-way fan-out.
    out_perm = out_t.ap().rearrange("b j h m -> j h b m")

    with TileContext(nc) as tc:
        with (
            tc.tile_pool(name="ins", bufs=1) as ipool,
            tc.tile_pool(name="outs", bufs=3) as opool,
        ):
            left_sb = ipool.tile([P, WC], bf16, tag="left")
            rpad_sb = ipool.tile([P, TC], bf16, tag="rpad")
            vrep_sb = ipool.tile([P, TC], bf16, tag="vrep")
            # Phased input loads: the head (~3.4 MB) drains alone at full read
            # bandwidth so the first output DMA can start early; the tail
            # halves are gated to drain underneath the first output DMAs
            # (without the gate, all loads round-robin on the shared SDMA
            # engines at packet granularity and the head finishes no earlier
            # than the whole input set). vrep is the 0/1 validity mask
            # pre-expanded across the 32 channels on host — a step-0
            # broadcast AP would drop the mul to DVE 1x mode (4.4us/op);
            # with a contiguous in1 it runs in 2x_1P packed mode.
            SPLIT_L = WCHUNK * C  # left head: w < 128 (everything wi=0 needs)
            SPLIT_R = 144 * C     # rpad/vrep head: t < 144 (wi=0 reads t < 140)
            head = [
                nc.sync.dma_start(out=vrep_sb[:, :SPLIT_R], in_=vrep_t[:, :SPLIT_R]),
                nc.sync.dma_start(out=left_sb[:, :SPLIT_L], in_=left_t[:, :SPLIT_L]),
                nc.sync.dma_start(out=rpad_sb[:, :SPLIT_R], in_=rpad_t[:, :SPLIT_R]),
            ]
            tail = [
                nc.scalar.dma_start(out=left_sb[:, SPLIT_L:], in_=left_t[:, SPLIT_L:]),
                nc.scalar.dma_start(out=rpad_sb[:, SPLIT_R:], in_=rpad_t[:, SPLIT_R:]),
                nc.scalar.dma_start(out=vrep_sb[:, SPLIT_R:], in_=vrep_t[:, SPLIT_R:]),
            ]
            for t_ in tail:
                for h_ in head:
                    add_dep_helper(
                        t_.ins, h_.ins,
                        reason="input tail loads drain after head loads",
                    )

            lv = left_sb[:].rearrange("p (w c) -> p w c", c=C)
            rv = rpad_sb[:].rearrange("p (t c) -> p t c", c=C)
            vv = vrep_sb[:].rearrange("p (t c) -> p t c", c=C)

            for wi in range(0, W, WCHUNK):
                for j in reversed(range(DPC)):
                    ot = opool.tile([P, WCHUNK * 2 * C], bf16, tag="ot")
                    ov = ot[:].rearrange("p (w c) -> p w c", c=2 * C)
                    t0 = wi + JPAD - j
                    nc.vector.tensor_mul(
                        out=ov[:, :, 0:C],
                        in0=lv[:, wi : wi + WCHUNK, :],
                        in1=vv[:, t0 : t0 + WCHUNK, :],
                    )
                    nc.vector.tensor_copy(
                        out=ov[:, :, C : 2 * C],
                        in_=rv[:, t0 : t0 + WCHUNK, :],
                    )
                    nc.sync.dma_start(
                        out=out_perm[j, :, :, wi * 2 * C : (wi + WCHUNK) * 2 * C],
                        in_=ot[:],
                    )
    nc.finalize()
    return nc


def get_nc():
    if "nc" not in _CACHE:
        _CACHE["nc"] = _build_nc()
    return _CACHE["nc"]


def _hb_major(x):
    """[B, H, rest...] -> [128 = (h, b) h-major, prod(rest)] contiguous."""
    return np.ascontiguousarray(x.transpose(1, 0, 2, 3)).reshape(P, -1)


def prep_inputs(left, right):
    """Build the 8 per-core input maps from full left/right."""
    left = np.asarray(left, dtype=F32).astype(BF16)
    right = np.asarray(right, dtype=F32).astype(BF16)
    left_flat = _hb_major(left)
    in_maps = []
    for k in range(N_CORES):
        d0 = DPC * k - MAX_DISP
        shift = JPAD + d0        # rpad[..., t, :] = right[..., t - shift, :]
        rpad = np.zeros((B, H, TPAD, C), BF16)
        lo, hi = max(0, shift), min(TPAD, shift + W)
        if lo < hi:
            rpad[:, :, lo:hi, :] = right[:, :, lo - shift : hi - shift, :]
        vk = np.zeros(TPAD, BF16)
        vk[lo:hi] = 1.0
        vrep = np.ascontiguousarray(
            np.broadcast_to(vk[None, :, None], (P, TPAD, C))
        ).reshape(P, TC)
        in_maps.append(
            {"left_flat": left_flat, "rpad": _hb_major(rpad), "vrep": vrep}
        )
    return in_maps


def run(left, right, **kwargs):
    """Run the SPMD kernel; returns (full_output, BassKernelResults)."""
    from concourse.bass_utils import run_bass_kernel_spmd

    nc = get_nc()
    in_maps = prep_inputs(left, right)
    try:
        res = run_bass_kernel_spmd(
            nc, in_maps, core_ids=list(range(N_CORES)), **kwargs
        )
    except Exception:
        # The axon/neuron device occasionally reports a transient
        # NRT_EXEC_UNIT_UNRECOVERABLE on a cold first run; a retry succeeds.
        res = run_bass_kernel_spmd(
            nc, in_maps, core_ids=list(range(N_CORES)), **kwargs
        )
    full = np.concatenate(
        [r["out"].reshape(B, DPC, H, W, 2 * C) for r in res.results], axis=1
    ).astype(np.float32)
    return full, res


def kernel(left, right):
    full, _ = run(left, right)
    return full



# revision 19
# speedup vs baseline: 1.1939x; 1.0316x over previous
# Cost-volume concatenation kernel for Trainium2 (Bass/Tile), SPMD over 8 cores.
#
# Problem: left, right: [B=2, H=64, W=256, C=32] f32.
# out[b, d+48, h, w, :32] = left[b,h,w,:]  * valid(w,d)
# out[b, d+48, h, w, 32:] = right[b,h,w-d,:] * valid(w,d),  d in [-48, 48)
# valid(w,d) = 0 <= w-d < W.  Output [2, 96, 64, 256, 64] f32 (~805 MB).
#
# Sharding: disparity axis. Core k handles the 12 levels d in [12k-48, 12k-36).
# The kernel program is identical on every core; all per-core variation lives in
# the DATA:
#   - rpad:  right pre-shifted by the core's base disparity and zero-padded to
#            width TPAD, so the in-kernel shift is j in [0,12) for every core and
#            the zero padding implements the right-half validity masking.
#   - vrep:  a 0/1 validity mask with the same index structure, replicated
#            across the 128 SBUF partitions; out_left = left * vrep_shifted
#            implements the left-half masking.
#
# SBUF layout: partitions = (h, b) — h-major — p = 2*h + b, 128 partitions;
# free dim = (w, c). h-major matters: the output DMA's DRAM access pattern is
# then [h=64, b=2, wc] with outer dim 64, which HWDGE fans out across all 16
# SDMA engines. (A b-major [2, 64, wc] pattern splits over only 2 engines ->
# ~27 GB/s per core; SWDGE spreads by partition but its descriptor ring
# backpressure caps concurrency at ~4 engines for multi-descriptor transfers.)
#
# Per disparity j the kernel assembles interleaved [left|right] rows in SBUF
# (two tensor ops per w-chunk) and streams them out with contiguous HWDGE DMAs.
#
# Precision: the whole pipeline runs in bf16 (inputs are rounded to bf16 on
# host; the mask is exact 0/1 and the copy/mul are exact in bf16, so the only
# error is the input rounding, ~1e-3 rel — far inside the 2e-2 gate). This
# halves DMA traffic vs f32: ~4.4 MB read + ~50 MB write per core
# (memory-bound; the f32 version measured 270us, bound at ~405 GB/s/core).

import ml_dtypes
import numpy as np

B, H, W, C = 2, 64, 256, 32
MAX_DISP = 48
D2 = 2 * MAX_DISP            # 96 disparity levels
N_CORES = 8
DPC = D2 // N_CORES          # 12 disparities per core
JPAD = DPC - 1               # 11: shift offset so in-kernel shifts are >= 0
TPAD = 272                   # padded t-width (>= W + JPAD = 267)
P = B * H                    # 128 SBUF partitions = (h, b) h-major
WC = W * C                   # 8192
TC = TPAD * C                # 8704
WCHUNK = 128                 # w-columns per output tile / DMA (2 MB per DMA)
F32 = np.float32
BF16 = ml_dtypes.bfloat16

_CACHE = {}


def _build_nc():
    import concourse.bacc as bacc
    import concourse.mybir as mybir
    from concourse.tile import TileContext, add_dep_helper

    bf16 = mybir.dt.bfloat16
    nc = bacc.Bacc("TRN2", target_bir_lowering=False, debug=False)
    left_t = nc.dram_tensor("left_flat", [P, WC], bf16, kind="ExternalInput")
    rpad_t = nc.dram_tensor("rpad", [P, TC], bf16, kind="ExternalInput")
    vrep_t = nc.dram_tensor("vrep", [P, TPAD], bf16, kind="ExternalInput")
    out_t = nc.dram_tensor("out", [B, DPC, H, W * 2 * C], bf16, kind="ExternalOutput")
    # DMA-side view iterating (j, h, b, cols): outer dim 64 for 16-way fan-out.
    out_perm = out_t.ap().rearrange("b j h m -> j h b m")

    with TileContext(nc) as tc:
        with (
            tc.tile_pool(name="ins", bufs=1) as ipool,
            tc.tile_pool(name="outs", bufs=3) as opool,
        ):
            left_sb = ipool.tile([P, WC], bf16, tag="left")
            rpad_sb = ipool.tile([P, TC], bf16, tag="rpad")
            vnar_sb = ipool.tile([P, TPAD], bf16, tag="vnar")
            vexp_sb = ipool.tile([P, TC], bf16, tag="vexp")
            # Phased input loads: the head (~2.4 MB) drains alone at full read
            # bandwidth so the first output DMA can start early; the tail
            # halves are gated to drain underneath the first output DMAs
            # (without the gate, all loads round-robin on the shared SDMA
            # engines at packet granularity and the head finishes no earlier
            # than the whole input set). The 0/1 validity mask is loaded
            # narrow ([P, TPAD], 139 KB) and expanded across the 32 channels
            # on-device by DVE broadcast-copies: the SBUF AXI fabric
            # (16 ports, ~435 GB/s) is the binding resource, so every input
            # byte not DMA'd is won back; meanwhile the mul must read a
            # channel-expanded contiguous mask (a step-0 broadcast in1 would
            # drop it from 2x_1P to 1x mode, 4.4us/op vs 2.3us/op).
            TSPLIT = 144          # rpad/vexp head in t cols (wi=0 reads t < 140)
            SPLIT_L = WCHUNK * C  # left head: w < 128 (everything wi=0 needs)
            SPLIT_R = TSPLIT * C
            head = [
                nc.sync.dma_start(out=vnar_sb[:], in_=vrep_t[:]),
                nc.sync.dma_start(out=left_sb[:, :SPLIT_L], in_=left_t[:, :SPLIT_L]),
                nc.sync.dma_start(out=rpad_sb[:, :SPLIT_R], in_=rpad_t[:, :SPLIT_R]),
            ]
            tail = [
                nc.scalar.dma_start(out=left_sb[:, SPLIT_L:], in_=left_t[:, SPLIT_L:]),
                nc.scalar.dma_start(out=rpad_sb[:, SPLIT_R:], in_=rpad_t[:, SPLIT_R:]),
            ]
            for t_ in tail:
                for h_ in head:
                    add_dep_helper(
                        t_.ins, h_.ins,
                        reason="input tail loads drain after head loads",
                    )

            lv = left_sb[:].rearrange("p (w c) -> p w c", c=C)
            rv = rpad_sb[:].rearrange("p (t c) -> p t c", c=C)
            vn = vnar_sb[:]  # [p, t]
            vv = vexp_sb[:].rearrange("p (t c) -> p t c", c=C)

            for wi in range(0, W, WCHUNK):
                # Expand the t-range of the mask this wi-block reads,
                # just-in-time so the first output tile isn't gated on the
                # whole expansion.
                tlo, thi = (0, TSPLIT) if wi == 0 else (TSPLIT, TPAD)
                nc.vector.tensor_copy(
                    out=vv[:, tlo:thi, :],
                    in_=vn[:, tlo:thi, None].broadcast_to([P, thi - tlo, C]),
                )
                for j in reversed(range(DPC)):
                    ot = opool.tile([P, WCHUNK * 2 * C], bf16, tag="ot")
                    ov = ot[:].rearrange("p (w c) -> p w c", c=2 * C)
                    t0 = wi + JPAD - j
                    nc.vector.tensor_mul(
                        out=ov[:, :, 0:C],
                        in0=lv[:, wi : wi + WCHUNK, :],
                        in1=vv[:, t0 : t0 + WCHUNK, :],
                    )
                    nc.vector.tensor_copy(
                        out=ov[:, :, C : 2 * C],
                        in_=rv[:, t0 : t0 + WCHUNK, :],
                    )
                    nc.sync.dma_start(
                        out=out_perm[j, :, :, wi * 2 * C : (wi + WCHUNK) * 2 * C],
                        in_=ot[:],
                    )
    nc.finalize()
    return nc


def get_nc():
    if "nc" not in _CACHE:
        _CACHE["nc"] = _build_nc()
    return _CACHE["nc"]


def _hb_major(x):
    """[B, H, rest...] -> [128 = (h, b) h-major, prod(rest)] contiguous."""
    return np.ascontiguousarray(x.transpose(1, 0, 2, 3)).reshape(P, -1)


def prep_inputs(left, right):
    """Build the 8 per-core input maps from full left/right."""
    left = np.asarray(left, dtype=F32).astype(BF16)
    right = np.asarray(right, dtype=F32).astype(BF16)
    left_flat = _hb_major(left)
    in_maps = []
    for k in range(N_CORES):
        d0 = DPC * k - MAX_DISP
        shift = JPAD + d0        # rpad[..., t, :] = right[..., t - shift, :]
        rpad = np.zeros((B, H, TPAD, C), BF16)
        lo, hi = max(0, shift), min(TPAD, shift + W)
        if lo < hi:
            rpad[:, :, lo:hi, :] = right[:, :, lo - shift : hi - shift, :]
        vk = np.zeros(TPAD, BF16)
        vk[lo:hi] = 1.0
        vrep = np.ascontiguousarray(np.broadcast_to(vk, (P, TPAD)))
        in_maps.append(
            {"left_flat": left_flat, "rpad": _hb_major(rpad), "vrep": vrep}
        )
    return in_maps


def run(left, right, **kwargs):
    """Run the SPMD kernel; returns (full_output, BassKernelResults)."""
    from concourse.bass_utils import run_bass_kernel_spmd

    nc = get_nc()
    in_maps = prep_inputs(left, right)
    try:
        res = run_bass_kernel_spmd(
            nc, in_maps, core_ids=list(range(N_CORES)), **kwargs
        )
    except Exception:
        # The axon/neuron device occasionally reports a transient
        # NRT_EXEC_UNIT_UNRECOVERABLE on a cold first run; a retry succeeds.
        res = run_bass_kernel_spmd(
            nc, in_maps, core_ids=list(range(N_CORES)), **kwargs
        )
    full = np.concatenate(
        [r["out"].reshape(B, DPC, H, W, 2 * C) for r in res.results], axis=1
    ).astype(np.float32)
    return full, res


def kernel(left, right):
    full, _ = run(left, right)
    return full

